# Initial kernel scaffold
#
"""Trainium2 Bass kernel for the ANIMA-Apex recurrent cell.

Math (per timestep, see reference):
    xe = tanh(x_t @ W_enc + b)
    W_new = tanh(xe + W@Ww + I@Wi + A@Wa) * sig([I,A]@Wg + b)
    z,r  = sig([W_new,I,A]@Iz/r + b)
    h    = tanh([W_new, r*I, A]@Ih + b)
    I_new = (1-z)*I + z*h
    cg   = sig([W_new,I_new]@c + b); branched = cg*then + (1-cg)*else (then/else = tanh(I_new@...))
    A_new = tanh(W_new@Aw + I_new@Ai + A@Aa + branched) * sig([W_new,I_new]@Ag + b)
    out  = A_new @ phi + b

Implementation notes:
  - Pure data parallelism: batch 1024 -> 128 per core across 8 cores.
  - States kept transposed [D, B] on SBUF partitions, stacked [I; W; A] in
    one [48, 128] bf16 tile so each group of matmuls is ONE TensorE
    instruction (weights (in,out) are directly lhsT; contraction over
    partitions).
  - All sigmoids computed as tanh half-angle: sig(x) = (tanh(x/2)+1)/2 with
    the 1/2 folded into weights/biases host-side, so every ACT stage is a
    single Tanh instruction over the stacked pre-activations (one act
    table, fewer ACT ops - ACT is the bottleneck engine).
  - W and A states stored as 2x their true value ((t+1)*t form needs no
    extra scale op); consumers' weight rows are pre-halved host-side.
  - branched = cg*T + (1-cg)*Eo enters the A_input PSUM via two identity
    matmuls with +-0.5*I so no extra DVE ops on the critical path.
  - bf16 everywhere on the matmul path (1 cycle/row on PE); fp32 PSUM.
    Measured end-to-end rel err vs fp32 reference ~4e-3.
"""

import numpy as np
import ml_dtypes

BF16 = ml_dtypes.bfloat16
B, T, S, D, O = 1024, 2048, 8, 4 + 12, 4
NCORES = 8
BC = B // NCORES          # 128 batch rows per core
E = 64                    # timesteps per chunk (loop body unroll)
ENC_N = 4                 # timesteps per encoder matmul (N = 4*128 = 512)


def _f32(a):
    return np.ascontiguousarray(np.asarray(a, dtype=np.float32))


def _bf(a):
    return np.ascontiguousarray(np.asarray(a, dtype=np.float32).astype(BF16))


def _prep_params(params):
    """Host-side weight packing. State row order is [I(0:16), W(16:32), A(32:48)].
    Stored states: I true, W and A stored as 2x true -> their weight rows halved.
    Sigmoid columns additionally halved (tanh half-angle)."""
    p = {k: _f32(v) for k, v in params.items()}
    Z = np.zeros((D, D), np.float32)

    def stack_iwa(wi, ww, wa):
        # rows for [I; W; A] rhs blocks, with state scaling
        return np.concatenate([wi, 0.5 * ww, 0.5 * wa], axis=0)

    # mmA: cols 0:16 = W_input partial, cols 16:32 = gate (halved)
    mma = np.concatenate(
        [
            stack_iwa(p["W_from_I"], p["W_from_W"], p["W_from_A"]),
            0.5 * np.concatenate([p["W_gate_w"][:D], Z, 0.5 * p["W_gate_w"][D:]], 0),
        ],
        axis=1,
    )
    # zr: z cols then r cols; reference row order [W_new, I, A] -> ours [I, W, A]
    def zr_block(w):
        return 0.5 * np.concatenate([w[D : 2 * D], 0.5 * w[:D], 0.5 * w[2 * D :]], 0)

    zr = np.concatenate([zr_block(p["I_z_w"]), zr_block(p["I_r_w"])], axis=1)
    # h: rows 0:32 for rhs STATE[16:48]=[W;A], rows 32:48 for rhs rI2 (=2 r*I)
    hw = np.concatenate(
        [0.5 * p["I_h_w"][:D], 0.5 * p["I_h_w"][2 * D :], 0.5 * p["I_h_w"][D : 2 * D]], 0
    )
    # cgte: rhs STATE[0:32] = [I_new; W_new(2x)]
    cvec = np.concatenate([p["cond_w"][D:], 0.5 * p["cond_w"][:D]], 0)  # (32,1)
    cg_b = 0.5 * np.repeat(cvec, D, axis=1)  # (32,16) broadcast trick
    ag = 0.5 * np.concatenate([p["A_gate_w"][D:], 0.5 * p["A_gate_w"][:D]], 0)
    thn = np.concatenate([p["then_w"], Z], 0)
    els = np.concatenate([p["else_w"], Z], 0)
    cgte = np.concatenate([cg_b, ag, thn, els], axis=1)  # (32, 64)
    ain = stack_iwa(p["A_from_I"], p["A_from_W"], p["A_from_A"])
    ident = np.eye(D, dtype=np.float32)
    ids = np.concatenate([ident, 0.5 * ident, -0.5 * ident], axis=1)  # (16,48)

    bias = np.zeros((64, 6), np.float32)
    bias[D : 2 * D, 0] = 0.5 * p["W_gate_b"]          # tanhA (rows 0:16 Wi: no bias)
    bias[:D, 1] = 0.5 * p["I_z_b"]
    bias[D : 2 * D, 1] = 0.5 * p["I_r_b"]
    bias[:D, 2] = p["I_h_b"]
    bias[:D, 3] = 0.5 * p["cond_b"][0]                # broadcast rows
    bias[D : 2 * D, 3] = 0.5 * p["A_gate_b"]
    bias[2 * D : 3 * D, 3] = p["then_b"]
    bias[3 * D : 4 * D, 3] = p["else_b"]
    bias[:D, 4] = p["W_enc_b"]
    phib = np.tile(p["phi_b"][None, :], (BC, 1)).astype(np.float32)

    return {
        "w_enc": _bf(p["W_enc_w"]),
        "w_mma": _bf(mma),
        "w_zr": _bf(zr),
        "w_h": _bf(hw),
        "w_cgte": _bf(cgte),
        "w_ain": _bf(ain),
        "w_ids": _bf(ids),
        "w_phi": _bf(0.5 * p["phi_w"]),
        "bias": _f32(bias),
        "phib": _f32(phib),
    }


_PARAM_SHAPES = {
    "w_enc": (S, D),
    "w_mma": (3 * D, 2 * D),
    "w_zr": (3 * D, 2 * D),
    "w_h": (3 * D, D),
    "w_cgte": (2 * D, 4 * D),
    "w_ain": (3 * D, D),
    "w_ids": (D, 3 * D),
    "w_phi": (D, O),
    "bias": (64, 6),
    "phib": (BC, O),
}


def _build():
    import concourse.bass as bass
    import concourse.mybir as mybir
    import concourse.tile as tile
    from concourse import bacc
    from concourse.bass import ds

    f32 = mybir.dt.float32
    bf16 = mybir.dt.bfloat16
    TANH = mybir.ActivationFunctionType.Tanh
    ADD = mybir.AluOpType.add
    SUB = mybir.AluOpType.subtract
    MUL = mybir.AluOpType.mult

    nc = bacc.Bacc("TRN2", target_bir_lowering=False, num_devices=NCORES)

    xin = nc.declare_dram_parameter("x", [S, T, BC], bf16, isOutput=False)
    wp = {}
    for k, shp in _PARAM_SHAPES.items():
        dt = f32 if k in ("bias", "phib") else bf16
        wp[k] = nc.declare_dram_parameter(k, list(shp), dt, isOutput=False)
    out = nc.declare_dram_parameter("out", [BC, T, O], f32, isOutput=True)

    with tile.TileContext(nc) as tc:
        with (
            tc.tile_pool(name="singles", bufs=1) as singles,
            tc.tile_pool(name="xin_p", bufs=2) as xin_p,
            tc.tile_pool(name="xe_p", bufs=2) as xe_p,
            tc.tile_pool(name="out_p", bufs=2) as out_p,
            tc.tile_pool(name="tmp", bufs=3) as tmp,
            tc.tile_pool(name="pA", bufs=1, space="PSUM") as pA,
            tc.tile_pool(name="pZR", bufs=1, space="PSUM") as pZR,
            tc.tile_pool(name="pH", bufs=1, space="PSUM") as pH,
            tc.tile_pool(name="pCT", bufs=1, space="PSUM") as pCT,
            tc.tile_pool(name="pAI", bufs=1, space="PSUM") as pAI,
            tc.tile_pool(name="pENC", bufs=2, space="PSUM") as pENC,
            tc.tile_pool(name="pPHI", bufs=1, space="PSUM") as pPHI,
        ):
            w = {}
            for k, shp in _PARAM_SHAPES.items():
                dt = f32 if k in ("bias", "phib") else bf16
                w[k] = singles.tile(list(shp), dt)
                nc.sync.dma_start(out=w[k][:], in_=wp[k][:])
            state = singles.tile([3 * D, BC], bf16)   # rows [I; W(2x); A(2x)]
            nc.vector.memset(state[:], 0.0)
            bias = w["bias"]

            with tc.For_i(0, T, E) as it:
                xt = xin_p.tile([S, E, BC], bf16)
                nc.sync.dma_start(out=xt[:], in_=xin[:, ds(it, E), :])
                ot = out_p.tile([BC, E, O], f32)
                xe = xe_p.tile([D, E * BC], bf16)

                # encoder for the whole chunk, ENC_N steps per matmul
                for j in range(E // ENC_N):
                    pe = pENC.tile([D, ENC_N * BC], f32)
                    nc.tensor.matmul(
                        pe[:],
                        w["w_enc"][:],
                        xt[:, j * ENC_N : (j + 1) * ENC_N, :].reshape(S, ENC_N * BC),
                        start=True,
                        stop=True,
                    )
                    nc.scalar.activation(
                        out=xe[:, j * ENC_N * BC : (j + 1) * ENC_N * BC],
                        in_=pe[:],
                        func=TANH,
                        bias=bias[:D, 4:5],
                    )

                for k in range(E):
                    # --- W stage ---
                    pa = pA.tile([2 * D, BC], f32)
                    nc.tensor.matmul(pa[:], w["w_mma"][:], state[:], start=True, stop=False)
                    nc.tensor.matmul(
                        pa[:D, :],
                        w["w_ids"][:, :D],
                        xe[:, k * BC : (k + 1) * BC],
                        start=False,
                        stop=True,
                        skip_group_check=True,
                    )
                    tt = tmp.tile([2 * D, BC], bf16, tag="tt")
                    nc.scalar.activation(out=tt[:], in_=pa[:], func=TANH, bias=bias[: 2 * D, 0:1])
                    # W_new(2x) = (tanh(g/2)+1)*tanh(Wi)
                    nc.vector.scalar_tensor_tensor(
                        out=state[D : 2 * D, :], in0=tt[D:, :], scalar=1.0, in1=tt[:D, :],
                        op0=ADD, op1=MUL,
                    )
                    # --- z/r stage ---
                    pz = pZR.tile([2 * D, BC], f32)
                    nc.tensor.matmul(pz[:], w["w_zr"][:], state[:], start=True, stop=True)
                    tz = tmp.tile([2 * D, BC], bf16, tag="tz")
                    nc.scalar.activation(out=tz[:], in_=pz[:], func=TANH, bias=bias[: 2 * D, 1:2])
                    ri2 = tmp.tile([D, BC], bf16, tag="ri2")  # 2*r*I
                    nc.vector.scalar_tensor_tensor(
                        out=ri2[:], in0=tz[D:, :], scalar=1.0, in1=state[:D, :],
                        op0=ADD, op1=MUL,
                    )
                    # --- h stage ---
                    ph = pH.tile([D, BC], f32)
                    nc.tensor.matmul(ph[:], w["w_h"][: 2 * D, :], state[D:, :], start=True, stop=False)
                    nc.tensor.matmul(ph[:], w["w_h"][2 * D :, :], ri2[:], start=False, stop=True,
                                     skip_group_check=True)
                    th = tmp.tile([D, BC], bf16, tag="th")
                    nc.scalar.activation(out=th[:], in_=ph[:], func=TANH, bias=bias[:D, 2:3])
                    # I_new = I + ((z'+1)*(h-I))/2
                    dd = tmp.tile([D, BC], bf16, tag="dd")
                    nc.vector.tensor_sub(dd[:], th[:], state[:D, :])
                    ee = tmp.tile([D, BC], bf16, tag="ee")
                    nc.vector.scalar_tensor_tensor(
                        out=ee[:], in0=tz[:D, :], scalar=1.0, in1=dd[:], op0=ADD, op1=MUL
                    )
                    nc.vector.scalar_tensor_tensor(
                        out=state[:D, :], in0=ee[:], scalar=0.5, in1=state[:D, :],
                        op0=MUL, op1=ADD,
                    )
                    # --- cond/gate/then/else stage ---
                    pc = pCT.tile([4 * D, BC], f32)
                    nc.tensor.matmul(pc[:], w["w_cgte"][:], state[: 2 * D, :], start=True, stop=True)
                    tc4 = tmp.tile([4 * D, BC], bf16, tag="tc4")
                    nc.scalar.activation(out=tc4[:], in_=pc[:], func=TANH, bias=bias[: 4 * D, 3:4])
                    uu = tmp.tile([D, BC], bf16, tag="uu")  # (c'+1)*T = 2 cg T
                    nc.vector.scalar_tensor_tensor(
                        out=uu[:], in0=tc4[:D, :], scalar=1.0, in1=tc4[2 * D : 3 * D, :],
                        op0=ADD, op1=MUL,
                    )
                    vv = tmp.tile([D, BC], bf16, tag="vv")  # (c'-1)*Eo = -2(1-cg) Eo
                    nc.vector.scalar_tensor_tensor(
                        out=vv[:], in0=tc4[:D, :], scalar=1.0, in1=tc4[3 * D :, :],
                        op0=SUB, op1=MUL,
                    )
                    # --- A stage ---
                    pai = pAI.tile([D, BC], f32)
                    nc.tensor.matmul(pai[:], w["w_ain"][:], state[:], start=True, stop=False)
                    nc.tensor.matmul(pai[:], w["w_ids"][:, D : 2 * D], uu[:], start=False,
                                     stop=False, skip_group_check=True)
                    nc.tensor.matmul(pai[:], w["w_ids"][:, 2 * D :], vv[:], start=False,
                                     stop=True, skip_group_check=True)
                    ta = tmp.tile([D, BC], bf16, tag="ta")
                    nc.scalar.activation(out=ta[:], in_=pai[:], func=TANH, bias=bias[:D, 5:6])
                    # A_new(2x) = (a'+1)*tanh(A_input)
                    nc.vector.scalar_tensor_tensor(
                        out=state[2 * D :, :], in0=tc4[D : 2 * D, :], scalar=1.0, in1=ta[:],
                        op0=ADD, op1=MUL,
                    )
                    # --- output ---
                    pp = pPHI.tile([BC, O], f32)
                    nc.tensor.matmul(pp[:], state[2 * D :, :], w["w_phi"][:], start=True, stop=True)
                    nc.vector.tensor_add(ot[:, k, :], pp[:], w["phib"][:])

                nc.sync.dma_start(out=out[:, ds(it, E), :], in_=ot[:])

    nc.finalize()
    return nc


def _run(nc, in_maps, trace):
    from concourse.bass_utils import run_bass_kernel_spmd

    if trace:
        import sys as _sys, types as _types

        try:
            import antenv.axon_hooks  # noqa: F401
        except ImportError:
            import trn_agent_boot.trn_boot as _tb

            _hook = _tb._ntff_profile_via_ctypes("/opt/axon/libaxon_pjrt.so")
            _m = _types.ModuleType("antenv.axon_hooks")
            _m.get_axon_ntff_profile_hook = lambda: _hook
            _sys.modules["antenv.axon_hooks"] = _m
    return run_bass_kernel_spmd(nc, in_maps, core_ids=list(range(NCORES)), trace=trace)


def kernel(x, params, _trace=False):
    x = np.asarray(x)
    assert x.shape == (B, T, S), x.shape
    pk = _prep_params(params)
    nc = _build()

    in_maps = []
    for c in range(NCORES):
        xc = np.ascontiguousarray(
            x[c * BC : (c + 1) * BC].transpose(2, 1, 0).astype(BF16)
        )  # (S, T, BC)
        m = {"x": xc}
        m.update(pk)
        in_maps.append(m)

    res = _run(nc, in_maps, _trace)
    kernel._last_results = res
    full = np.empty((B, T, O), np.float32)
    for c in range(NCORES):
        full[c * BC : (c + 1) * BC] = res.results[c]["out"]
    return full


# revision 9
# speedup vs baseline: 6.4650x; 6.4650x over previous
"""Trainium2 Bass kernel for the ANIMA-Apex recurrent cell (8-core data parallel).

Layout: states transposed [D, B], batch 128 per core. One persistent SBUF
tile STK [80, 128] bf16 holds W(2x) at rows 0:16, a constant-ones row at 16,
I at rows 32:48, A(2x) at rows 64:80 (zeros elsewhere). Row homes are chosen
so that:
  - every matmul operand sits at a partition base in {0,32,64} (PE rule),
  - every elementwise chain stays on one fixed 16-row range (DVE/ACT are
    partition-lane-locked: all operands of an op must share partitions),
  - biases ride for free as weight rows against the ones row.
Each stage's matmuls contract the full stacked state (K=80, zero rows padded
in the weights - free on the PE, time is N cycles) and write their 16-row
pre-activation block to the consuming chain's home rows, different outputs
side by side along the PSUM free dim, so one Tanh instruction activates a
whole stage. sig(x) = (tanh(x/2)+1)/2 with the 1/2 folded into weights
host-side; W/A states stored 2x so the (t'+1)*t forms need no extra scaling
ops; branched enters the A_input PSUM via two +-0.5-identity matmuls.
All matmul-path tensors bf16 (1 PE cycle/row); rel err vs f32 ref ~4e-3.
"""

import numpy as np
import ml_dtypes

BF16 = ml_dtypes.bfloat16
B, T, S, D, O = 1024, 2048, 8, 16, 4
NCORES = 8
BC = B // NCORES          # 128 batch rows per core
E = 64                    # timesteps per chunk (loop body unroll)
ENC_N = 4                 # timesteps per encoder matmul (N = 4*128 = 512)

# column index of each lhsT block inside the packed [80, 10*16] weight stack
WI, G, Z, R, H, CG, AG, TH, EL, AIN = range(10)


def _f32(a):
    return np.ascontiguousarray(np.asarray(a, dtype=np.float32))


def _bf(a):
    return np.ascontiguousarray(np.asarray(a, dtype=np.float32).astype(BF16))


def _prep_params(params):
    p = {k: _f32(v) for k, v in params.items()}

    def full(wW, wI, wA, b, sig):
        # lhsT [80, cols] against rhs STK rows [W(2x);1;.;I;.;A(2x)]
        cols = np.asarray(b).shape[-1] if np.ndim(b) else D
        L = np.zeros((80, cols), np.float32)
        s = 0.5 if sig else 1.0
        if wW is not None:
            L[0:D] = s * 0.5 * wW
        L[D] = s * np.asarray(b)
        if wI is not None:
            L[2 * D : 3 * D] = s * wI
        if wA is not None:
            L[4 * D : 5 * D] = s * 0.5 * wA
        return L

    cond_bc = np.repeat(p["cond_w"], D, axis=1)  # (32, 16) broadcast trick
    blocks = [None] * 10
    blocks[WI] = full(p["W_from_W"], p["W_from_I"], p["W_from_A"], np.zeros(D), False)
    blocks[G] = full(None, p["W_gate_w"][:D], p["W_gate_w"][D:], p["W_gate_b"], True)
    blocks[Z] = full(p["I_z_w"][:D], p["I_z_w"][D : 2 * D], p["I_z_w"][2 * D :], p["I_z_b"], True)
    blocks[R] = full(p["I_r_w"][:D], p["I_r_w"][D : 2 * D], p["I_r_w"][2 * D :], p["I_r_b"], True)
    blocks[H] = full(p["I_h_w"][:D], None, p["I_h_w"][2 * D :], p["I_h_b"], False)
    blocks[CG] = full(cond_bc[:D], cond_bc[D:], None, np.full(D, p["cond_b"][0]), True)
    blocks[AG] = full(p["A_gate_w"][:D], p["A_gate_w"][D:], None, p["A_gate_b"], True)
    blocks[TH] = full(None, p["then_w"], None, p["then_b"], False)
    blocks[EL] = full(None, p["else_w"], None, p["else_b"], False)
    blocks[AIN] = full(p["A_from_W"], p["A_from_I"], p["A_from_A"], np.zeros(D), False)
    wstk = np.concatenate(blocks, axis=1)  # (80, 160)

    hri = np.zeros((48, D), np.float32)
    hri[2 * D :] = 0.5 * p["I_h_w"][D : 2 * D]   # rI2 = 2*r*I -> halve
    ident = np.eye(D, dtype=np.float32)
    ids = np.zeros((80, 3 * D), np.float32)
    ids[:D, :D] = ident                           # xe inject (rows 0:16)
    ids[4 * D : 5 * D, D : 2 * D] = 0.5 * ident   # +0.5 u inject (rows 64:80)
    ids[4 * D : 5 * D, 2 * D :] = -0.5 * ident    # -0.5 v inject
    phi = np.zeros((80, O), np.float32)
    phi[4 * D : 5 * D] = 0.5 * p["phi_w"]         # A stored 2x
    enc = np.concatenate([p["W_enc_w"], p["W_enc_b"][None, :]], axis=0)  # (9,16)
    phib = np.tile(p["phi_b"][None, :], (BC, 1)).astype(np.float32)

    stk0 = np.zeros((80, BC), np.float32)
    stk0[D] = 1.0
    return {
        "stk0": _bf(stk0),
        "wstk": _bf(wstk),
        "w_hri": _bf(hri),
        "w_ids": _bf(ids),
        "w_phi": _bf(phi),
        "w_enc": _bf(enc),
        "phib": _f32(phib),
    }


_PARAM_SHAPES = {
    "stk0": (80, BC),
    "wstk": (80, 160),
    "w_hri": (48, D),
    "w_ids": (80, 3 * D),
    "w_phi": (80, O),
    "w_enc": (S + 1, D),
    "phib": (BC, O),
}


def _build(t_steps=T):
    import concourse.mybir as mybir
    import concourse.tile as tile
    from concourse import bacc
    from concourse.bass import ds

    f32 = mybir.dt.float32
    bf16 = mybir.dt.bfloat16
    TANH = mybir.ActivationFunctionType.Tanh
    ADD = mybir.AluOpType.add
    SUB = mybir.AluOpType.subtract
    MUL = mybir.AluOpType.mult

    nc = bacc.Bacc("TRN2", target_bir_lowering=False, num_devices=NCORES)

    xin = nc.declare_dram_parameter("x", [S + 1, t_steps, BC], bf16, isOutput=False)
    wp = {}
    for k, shp in _PARAM_SHAPES.items():
        dt = f32 if k == "phib" else bf16
        wp[k] = nc.declare_dram_parameter(k, list(shp), dt, isOutput=False)
    out = nc.declare_dram_parameter("out", [BC, t_steps, O], f32, isOutput=True)

    with tile.TileContext(nc) as tc:
        with (
            tc.tile_pool(name="singles", bufs=1) as singles,
            tc.tile_pool(name="xin_p", bufs=2) as xin_p,
            tc.tile_pool(name="xe_p", bufs=2) as xe_p,
            tc.tile_pool(name="out_p", bufs=2) as out_p,
            tc.tile_pool(name="tmp", bufs=3) as tmp,
            tc.tile_pool(name="pA", bufs=1, space="PSUM") as pA,
            tc.tile_pool(name="pZR", bufs=1, space="PSUM") as pZR,
            tc.tile_pool(name="pH", bufs=1, space="PSUM") as pH,
            tc.tile_pool(name="pCT", bufs=1, space="PSUM") as pCT,
            tc.tile_pool(name="pAI", bufs=1, space="PSUM") as pAI,
            tc.tile_pool(name="pENC", bufs=2, space="PSUM") as pENC,
            tc.tile_pool(name="pPHI", bufs=1, space="PSUM") as pPHI,
        ):
            w = {}
            for k, shp in _PARAM_SHAPES.items():
                dt = f32 if k == "phib" else bf16
                w[k] = singles.tile(list(shp), dt, name=f"w_{k}")
                nc.sync.dma_start(out=w[k][:], in_=wp[k][:])
            ws = w["wstk"]

            def blk(i):
                return ws[:, i * D : (i + 1) * D]

            # STK rows: W(2x) 0:16 | ones 16 | 0 | I 32:48 | 0 | A(2x) 64:80
            stk = singles.tile([80, BC], bf16)
            nc.sync.dma_start(out=stk[:], in_=wp["stk0"][:])
            ri2 = singles.tile([48, BC], bf16)     # rows 32:48 = 2*r*I
            nc.vector.memset(ri2[:], 0.0)

            with tc.For_i(0, t_steps, E) as it:
                xt = xin_p.tile([S + 1, E, BC], bf16)
                nc.sync.dma_start(out=xt[:], in_=xin[:, ds(it, E), :])
                ot = out_p.tile([BC, E, O], f32)
                xe = xe_p.tile([D, E * BC], bf16)

                for j in range(E // ENC_N):
                    pe = pENC.tile([D, ENC_N * BC], f32)
                    nc.tensor.matmul(
                        pe[:],
                        w["w_enc"][:],
                        xt[:, j * ENC_N : (j + 1) * ENC_N, :].rearrange("s e b -> s (e b)"),
                        start=True,
                        stop=True,
                    )
                    nc.scalar.activation(
                        out=xe[:, j * ENC_N * BC : (j + 1) * ENC_N * BC],
                        in_=pe[:], func=TANH,
                    )

                for k in range(E):
                    xek = xe[:, k * BC : (k + 1) * BC]
                    # --- W stage (home rows 0:16) ---
                    pa = pA.tile([D, 2 * BC], f32)
                    nc.tensor.matmul(pa[:, :BC], blk(WI), stk[:], start=True, stop=False)
                    nc.tensor.matmul(pa[:, :BC], w["w_ids"][:D, :D], xek,
                                     start=False, stop=True, skip_group_check=True)
                    nc.tensor.matmul(pa[:, BC:], blk(G), stk[:], start=True, stop=True)
                    tt = tmp.tile([D, 2 * BC], bf16, tag="tt")
                    nc.scalar.activation(out=tt[:], in_=pa[:], func=TANH)
                    # W_new(2x) = (tanh(g/2)+1)*tanh(Wi)
                    nc.vector.scalar_tensor_tensor(
                        out=stk[:D, :], in0=tt[:, BC:], scalar=1.0, in1=tt[:, :BC],
                        op0=ADD, op1=MUL)
                    # --- z/r stage (home rows 32:48) ---
                    pz = pZR.tile([48, 2 * BC], f32)
                    nc.tensor.matmul(pz[2 * D :, :BC], blk(Z), stk[:], start=True, stop=True)
                    nc.tensor.matmul(pz[2 * D :, BC:], blk(R), stk[:], start=True, stop=True)
                    tz = tmp.tile([48, 2 * BC], bf16, tag="tz")
                    nc.scalar.activation(out=tz[2 * D :, :], in_=pz[2 * D :, :], func=TANH)
                    # ri2 = (r'+1)*I_old = 2*r*I
                    nc.vector.scalar_tensor_tensor(
                        out=ri2[2 * D :, :], in0=tz[2 * D :, BC:], scalar=1.0,
                        in1=stk[2 * D : 3 * D, :], op0=ADD, op1=MUL)
                    # --- h stage (home rows 32:48) ---
                    ph = pH.tile([48, BC], f32)
                    nc.tensor.matmul(ph[2 * D :, :], blk(H), stk[:], start=True, stop=False)
                    nc.tensor.matmul(ph[2 * D :, :], w["w_hri"][2 * D :, :], ri2[2 * D :, :],
                                     start=False, stop=True, skip_group_check=True)
                    th = tmp.tile([48, BC], bf16, tag="th")
                    nc.scalar.activation(out=th[2 * D :, :], in_=ph[2 * D :, :], func=TANH)
                    # I_new = I + (z'+1)*(h-I)/2
                    dd = tmp.tile([48, BC], bf16, tag="dd")
                    nc.vector.tensor_sub(dd[2 * D :, :], th[2 * D :, :], stk[2 * D : 3 * D, :])
                    ee = tmp.tile([48, BC], bf16, tag="ee")
                    nc.vector.scalar_tensor_tensor(
                        out=ee[2 * D :, :], in0=tz[2 * D :, :BC], scalar=1.0,
                        in1=dd[2 * D :, :], op0=ADD, op1=MUL)
                    nc.vector.scalar_tensor_tensor(
                        out=stk[2 * D : 3 * D, :], in0=ee[2 * D :, :], scalar=0.5,
                        in1=stk[2 * D : 3 * D, :], op0=MUL, op1=ADD)
                    # --- cond/A_gate/then/else stage (home rows 64:80) ---
                    pc = pCT.tile([80, 4 * BC], f32)
                    for i, cb in enumerate((CG, AG, TH, EL)):
                        nc.tensor.matmul(pc[4 * D :, i * BC : (i + 1) * BC], blk(cb),
                                         stk[:], start=True, stop=True)
                    tc4 = tmp.tile([80, 4 * BC], bf16, tag="tc4")
                    nc.scalar.activation(out=tc4[4 * D :, :], in_=pc[4 * D :, :], func=TANH)
                    uu = tmp.tile([80, BC], bf16, tag="uu")   # (c'+1)*T = 2 cg T
                    nc.vector.scalar_tensor_tensor(
                        out=uu[4 * D :, :], in0=tc4[4 * D :, :BC], scalar=1.0,
                        in1=tc4[4 * D :, 2 * BC : 3 * BC], op0=ADD, op1=MUL)
                    vv = tmp.tile([80, BC], bf16, tag="vv")   # (c'-1)*Eo = -2(1-cg) Eo
                    nc.vector.scalar_tensor_tensor(
                        out=vv[4 * D :, :], in0=tc4[4 * D :, :BC], scalar=1.0,
                        in1=tc4[4 * D :, 3 * BC :], op0=SUB, op1=MUL)
                    # --- A stage (home rows 64:80) ---
                    pai = pAI.tile([80, BC], f32)
                    nc.tensor.matmul(pai[4 * D :, :], blk(AIN), stk[:], start=True, stop=False)
                    nc.tensor.matmul(pai[4 * D :, :], w["w_ids"][4 * D :, D : 2 * D],
                                     uu[4 * D :, :], start=False, stop=False,
                                     skip_group_check=True)
                    nc.tensor.matmul(pai[4 * D :, :], w["w_ids"][4 * D :, 2 * D :],
                                     vv[4 * D :, :], start=False, stop=True,
                                     skip_group_check=True)
                    ta = tmp.tile([80, BC], bf16, tag="ta")
                    nc.scalar.activation(out=ta[4 * D :, :], in_=pai[4 * D :, :], func=TANH)
                    # A_new(2x) = (a'+1)*tanh(A_input)
                    nc.vector.scalar_tensor_tensor(
                        out=stk[4 * D :, :], in0=tc4[4 * D :, BC : 2 * BC], scalar=1.0,
                        in1=ta[4 * D :, :], op0=ADD, op1=MUL)
                    # --- output ---
                    pp = pPHI.tile([BC, O], f32)
                    nc.tensor.matmul(pp[:], stk[4 * D :, :], w["w_phi"][4 * D :, :],
                                     start=True, stop=True)
                    nc.vector.tensor_add(ot[:, k, :], pp[:], w["phib"][:])

                nc.sync.dma_start(out=out[:, ds(it, E), :], in_=ot[:])

    nc.finalize()
    return nc


def _run(nc, in_maps, trace):
    from concourse.bass_utils import run_bass_kernel_spmd

    if trace:
        import sys as _sys, types as _types

        try:
            import antenv.axon_hooks  # noqa: F401
        except ImportError:
            import trn_agent_boot.trn_boot as _tb

            _hook = _tb._ntff_profile_via_ctypes("/opt/axon/libaxon_pjrt.so")
            _m = _types.ModuleType("antenv.axon_hooks")
            _m.get_axon_ntff_profile_hook = lambda: _hook
            _sys.modules["antenv.axon_hooks"] = _m
    return run_bass_kernel_spmd(nc, in_maps, core_ids=list(range(NCORES)), trace=trace)


def kernel(x, params, _trace=False, _t_steps=T):
    x = np.asarray(x)
    assert x.shape[0] == B and x.shape[2] == S, x.shape
    x = x[:, :_t_steps]
    pk = _prep_params(params)
    nc = _build(_t_steps)

    in_maps = []
    for c in range(NCORES):
        xc = x[c * BC : (c + 1) * BC].transpose(2, 1, 0)  # (S, t, BC)
        xc = np.concatenate([xc, np.ones((1, _t_steps, BC), np.float32)], axis=0)
        m = {"x": np.ascontiguousarray(xc.astype(BF16))}
        m.update(pk)
        in_maps.append(m)

    res = _run(nc, in_maps, _trace)
    kernel._last_results = res
    full = np.empty((B, _t_steps, O), np.float32)
    for c in range(NCORES):
        full[c * BC : (c + 1) * BC] = res.results[c]["out"]
    return full


# revision 12
# speedup vs baseline: 7.8925x; 1.2208x over previous
"""Trainium2 Bass kernel for the ANIMA-Apex recurrent cell (8-core data parallel).

Layout: states transposed [D, B], batch 128 per core. One persistent SBUF
tile STK [80, 128] bf16 holds W(2x) at rows 0:16, a constant-ones row at 16,
I at rows 32:48, A(2x) at rows 64:80 (zeros elsewhere). Row homes are chosen
so that:
  - every matmul operand sits at a partition base in {0,32,64} (PE rule),
  - every elementwise chain stays on one fixed 16-row range (DVE/ACT are
    partition-lane-locked: all operands of an op must share partitions),
  - biases ride for free as weight rows against the ones row.
Each stage's matmuls contract the full stacked state (K=80, zero rows padded
in the weights - free on the PE, time is N cycles) and write their 16-row
pre-activation block to the consuming chain's home rows, different outputs
side by side along the PSUM free dim, so one Tanh instruction activates a
whole stage. sig(x) = (tanh(x/2)+1)/2 with the 1/2 folded into weights
host-side; W/A states stored 2x so the (t'+1)*t forms need no extra scaling
ops; branched enters the A_input PSUM via two +-0.5-identity matmuls.
All matmul-path tensors bf16 (1 PE cycle/row); rel err vs f32 ref ~4e-3.
"""

import numpy as np
import ml_dtypes

BF16 = ml_dtypes.bfloat16
B, T, S, D, O = 1024, 2048, 8, 16, 4
NCORES = 8
BC = B // NCORES          # 128 batch rows per core
E = 32                    # timesteps per chunk (loop body unroll)
GB = 2                    # interleaved half-batch groups
BG = BC // GB             # 64 batch lanes per group
ENC_N = 4                 # timesteps per encoder matmul (N = 4*128 = 512)

# column index of each lhsT block inside the packed [80, 10*16] weight stack
WI, G, Z, R, H, CG, AG, TH, EL, AIN = range(10)


def _f32(a):
    return np.ascontiguousarray(np.asarray(a, dtype=np.float32))


def _bf(a):
    return np.ascontiguousarray(np.asarray(a, dtype=np.float32).astype(BF16))


def _prep_params(params):
    p = {k: _f32(v) for k, v in params.items()}

    def full(wW, wI, wA, b, sig):
        # lhsT [80, cols] against rhs STK rows [W(2x);1;.;I;.;A(2x)]
        cols = np.asarray(b).shape[-1] if np.ndim(b) else D
        L = np.zeros((80, cols), np.float32)
        s = 0.5 if sig else 1.0
        if wW is not None:
            L[0:D] = s * 0.5 * wW
        L[D] = s * np.asarray(b)
        if wI is not None:
            L[2 * D : 3 * D] = s * wI
        if wA is not None:
            L[4 * D : 5 * D] = s * 0.5 * wA
        return L

    cond_bc = np.repeat(p["cond_w"], D, axis=1)  # (32, 16) broadcast trick
    blocks = [None] * 10
    blocks[WI] = full(p["W_from_W"], p["W_from_I"], p["W_from_A"], np.zeros(D), False)
    blocks[G] = full(None, p["W_gate_w"][:D], p["W_gate_w"][D:], p["W_gate_b"], True)
    blocks[Z] = full(p["I_z_w"][:D], p["I_z_w"][D : 2 * D], p["I_z_w"][2 * D :], p["I_z_b"], True)
    blocks[R] = full(p["I_r_w"][:D], p["I_r_w"][D : 2 * D], p["I_r_w"][2 * D :], p["I_r_b"], True)
    blocks[H] = full(p["I_h_w"][:D], None, p["I_h_w"][2 * D :], p["I_h_b"], False)
    blocks[CG] = full(cond_bc[:D], cond_bc[D:], None, np.full(D, p["cond_b"][0]), True)
    blocks[AG] = full(p["A_gate_w"][:D], p["A_gate_w"][D:], None, p["A_gate_b"], True)
    blocks[TH] = full(None, p["then_w"], None, p["then_b"], False)
    blocks[EL] = full(None, p["else_w"], None, p["else_b"], False)
    blocks[AIN] = full(p["A_from_W"], p["A_from_I"], p["A_from_A"], np.zeros(D), False)
    wstk = np.concatenate(blocks, axis=1)  # (80, 160)

    hri = np.zeros((48, D), np.float32)
    hri[2 * D :] = 0.5 * p["I_h_w"][D : 2 * D]   # rI2 = 2*r*I -> halve
    ident = np.eye(D, dtype=np.float32)
    ids = np.zeros((80, 3 * D), np.float32)
    ids[:D, :D] = ident                           # xe inject (rows 0:16)
    ids[4 * D : 5 * D, D : 2 * D] = 0.5 * ident   # +0.5 u inject (rows 64:80)
    ids[4 * D : 5 * D, 2 * D :] = -0.5 * ident    # -0.5 v inject
    phi = np.zeros((80, O), np.float32)
    phi[4 * D : 5 * D] = 0.5 * p["phi_w"]         # A stored 2x
    enc = np.concatenate([p["W_enc_w"], p["W_enc_b"][None, :]], axis=0)  # (9,16)
    phib = np.tile(p["phi_b"][None, None, :], (BC, E, 1)).reshape(BC, E * O).astype(np.float32)

    stk0 = np.zeros((80, BC), np.float32)
    stk0[D] = 1.0
    return {
        "stk0": _bf(stk0),
        "wstk": _bf(wstk),
        "w_hri": _bf(hri),
        "w_ids": _bf(ids),
        "w_phi": _bf(phi),
        "w_enc": _bf(enc),
        "phib": _f32(phib),
    }


_PARAM_SHAPES = {
    "stk0": (80, BC),
    "wstk": (80, 160),
    "w_hri": (48, D),
    "w_ids": (80, 3 * D),
    "w_phi": (80, O),
    "w_enc": (S + 1, D),
    "phib": (BC, E * O),
}


def _build(t_steps=T):
    import concourse.mybir as mybir
    import concourse.tile as tile
    from concourse import bacc
    from concourse.bass import ds

    f32 = mybir.dt.float32
    bf16 = mybir.dt.bfloat16
    TANH = mybir.ActivationFunctionType.Tanh
    ADD = mybir.AluOpType.add
    SUB = mybir.AluOpType.subtract
    MUL = mybir.AluOpType.mult

    nc = bacc.Bacc("TRN2", target_bir_lowering=False, num_devices=NCORES)

    xin = nc.declare_dram_parameter("x", [S + 1, t_steps, BC], bf16, isOutput=False)
    wp = {}
    for k, shp in _PARAM_SHAPES.items():
        dt = f32 if k == "phib" else bf16
        wp[k] = nc.declare_dram_parameter(k, list(shp), dt, isOutput=False)
    out = nc.declare_dram_parameter("out", [BC, t_steps, O], f32, isOutput=True)

    with tile.TileContext(nc) as tc:
        with (
            tc.tile_pool(name="singles", bufs=1) as singles,
            tc.tile_pool(name="xin_p", bufs=2) as xin_p,
            tc.tile_pool(name="xe_p", bufs=2) as xe_p,
            tc.tile_pool(name="out_p", bufs=2) as out_p,
            tc.tile_pool(name="tmp", bufs=2) as tmp,
            tc.tile_pool(name="pST", bufs=2, space="PSUM") as pST,
            tc.tile_pool(name="pENC", bufs=2, space="PSUM") as pENC,
            tc.tile_pool(name="pPHI", bufs=2, space="PSUM") as pPHI,
        ):
            w = {}
            for k, shp in _PARAM_SHAPES.items():
                dt = f32 if k == "phib" else bf16
                w[k] = singles.tile(list(shp), dt, name=f"w_{k}")
                nc.sync.dma_start(out=w[k][:], in_=wp[k][:])
            ws = w["wstk"]

            def blk(i):
                return ws[:, i * D : (i + 1) * D]

            # per-group persistent state [80, BG]: W(2x) 0:16 | ones 16 | I 32:48 | A(2x) 64:80
            stks = []
            for g in range(GB):
                st = singles.tile([80, BG], bf16, name=f"stk{g}")
                nc.sync.dma_start(out=st[:], in_=wp["stk0"][:, g * BG : (g + 1) * BG])
                stks.append(st)

            D2, D4 = 2 * D, 4 * D
            with tc.For_i(0, t_steps, E) as it:
                xt = xin_p.tile([S + 1, E, BC], bf16)
                nc.sync.dma_start(out=xt[:], in_=xin[:, ds(it, E), :])
                ot = out_p.tile([BC, E * O], f32)
                xe = xe_p.tile([D, E * BC], bf16)

                for j in range(E // ENC_N):
                    pe = pENC.tile([D, ENC_N * BC], f32)
                    nc.tensor.matmul(
                        pe[:],
                        w["w_enc"][:],
                        xt[:, j * ENC_N : (j + 1) * ENC_N, :].rearrange("s e b -> s (e b)"),
                        start=True, stop=True,
                    )
                    nc.scalar.activation(
                        out=xe[:, j * ENC_N * BC : (j + 1) * ENC_N * BC],
                        in_=pe[:], func=TANH,
                    )

                pp = pPHI.tile([BC, E * O], f32)
                for k in range(E):
                    # one PSUM bank per group holds every stage block this step:
                    # wi [0:16,0:BG] | g [0:16,BG:2BG] | z [32:48,0:BG] | r [32:48,BG:2BG]
                    # h [32:48,2BG:3BG] | cg/ag/T/E [64:80,0:4BG] | ai [64:80,4BG:5BG]
                    ps = [pST.tile([80, 8 * BG], f32, name=f"ps{g}", tag=f"ps{g}") for g in range(GB)]
                    tt = [tmp.tile([D, 2 * BG], bf16, tag=f"tt{g}", name=f"tt{g}") for g in range(GB)]
                    tz = [tmp.tile([48, 2 * BG], bf16, tag=f"tz{g}", name=f"tz{g}") for g in range(GB)]
                    th = [tmp.tile([48, BG], bf16, tag=f"th{g}", name=f"th{g}") for g in range(GB)]
                    dd = [tmp.tile([48, BG], bf16, tag=f"dd{g}", name=f"dd{g}") for g in range(GB)]
                    ee = [tmp.tile([48, BG], bf16, tag=f"ee{g}", name=f"ee{g}") for g in range(GB)]
                    t4 = [tmp.tile([80, 4 * BG], bf16, tag=f"t4{g}", name=f"t4{g}") for g in range(GB)]
                    uu = [tmp.tile([80, BG], bf16, tag=f"uu{g}", name=f"uu{g}") for g in range(GB)]
                    vv = [tmp.tile([80, BG], bf16, tag=f"vv{g}", name=f"vv{g}") for g in range(GB)]
                    ta = [tmp.tile([80, BG], bf16, tag=f"ta{g}", name=f"ta{g}") for g in range(GB)]
                    ri = [tmp.tile([48, BG], bf16, tag=f"ri{g}", name=f"ri{g}") for g in range(GB)]

                    for g in range(GB):
                        xek = xe[:, k * BC + g * BG : k * BC + (g + 1) * BG]
                        nc.tensor.matmul(ps[g][:D, :BG], blk(WI), stks[g][:],
                                         start=True, stop=False, skip_group_check=True)
                        nc.tensor.matmul(ps[g][:D, :BG], w["w_ids"][:D, :D], xek,
                                         start=False, stop=True, skip_group_check=True)
                        nc.tensor.matmul(ps[g][:D, BG : 2 * BG], blk(G), stks[g][:],
                                         start=True, stop=True, skip_group_check=True)
                    for g in range(GB):
                        nc.scalar.activation(out=tt[g][:], in_=ps[g][:D, : 2 * BG], func=TANH)
                    for g in range(GB):  # W_new(2x) = (tanh(g/2)+1)*tanh(Wi)
                        nc.vector.scalar_tensor_tensor(
                            out=stks[g][:D, :], in0=tt[g][:, BG:], scalar=1.0,
                            in1=tt[g][:, :BG], op0=ADD, op1=MUL)
                    for g in range(GB):
                        nc.tensor.matmul(ps[g][D2 : 3 * D, :BG], blk(Z), stks[g][:],
                                         start=True, stop=True, skip_group_check=True)
                        nc.tensor.matmul(ps[g][D2 : 3 * D, BG : 2 * BG], blk(R), stks[g][:],
                                         start=True, stop=True, skip_group_check=True)
                    for g in range(GB):
                        nc.scalar.activation(out=tz[g][D2:, :], in_=ps[g][D2 : 3 * D, : 2 * BG],
                                             func=TANH)
                    for g in range(GB):  # ri = (r'+1)*I_old = 2*r*I
                        nc.vector.scalar_tensor_tensor(
                            out=ri[g][D2:, :], in0=tz[g][D2:, BG:], scalar=1.0,
                            in1=stks[g][D2 : 3 * D, :], op0=ADD, op1=MUL)
                    for g in range(GB):
                        nc.tensor.matmul(ps[g][D2 : 3 * D, 2 * BG : 3 * BG], blk(H), stks[g][:],
                                         start=True, stop=False, skip_group_check=True)
                        nc.tensor.matmul(ps[g][D2 : 3 * D, 2 * BG : 3 * BG],
                                         w["w_hri"][D2:, :], ri[g][D2:, :],
                                         start=False, stop=True, skip_group_check=True)
                    for g in range(GB):
                        nc.scalar.activation(out=th[g][D2:, :],
                                             in_=ps[g][D2 : 3 * D, 2 * BG : 3 * BG], func=TANH)
                    for g in range(GB):  # I_new = I + (z'+1)*(h-I)/2
                        nc.vector.tensor_sub(dd[g][D2:, :], th[g][D2:, :], stks[g][D2 : 3 * D, :])
                    for g in range(GB):
                        nc.vector.scalar_tensor_tensor(
                            out=ee[g][D2:, :], in0=tz[g][D2:, :BG], scalar=1.0,
                            in1=dd[g][D2:, :], op0=ADD, op1=MUL)
                    for g in range(GB):
                        nc.vector.scalar_tensor_tensor(
                            out=stks[g][D2 : 3 * D, :], in0=ee[g][D2:, :], scalar=0.5,
                            in1=stks[g][D2 : 3 * D, :], op0=MUL, op1=ADD)
                    for g in range(GB):
                        for i, cb in enumerate((CG, AG, TH, EL)):
                            nc.tensor.matmul(ps[g][D4:, i * BG : (i + 1) * BG], blk(cb),
                                             stks[g][:], start=True, stop=True,
                                             skip_group_check=True)
                    for g in range(GB):
                        nc.scalar.activation(out=t4[g][D4:, :], in_=ps[g][D4:, : 4 * BG],
                                             func=TANH)
                    for g in range(GB):  # uu = (c'+1)*T; vv = (c'-1)*Eo
                        nc.vector.scalar_tensor_tensor(
                            out=uu[g][D4:, :], in0=t4[g][D4:, :BG], scalar=1.0,
                            in1=t4[g][D4:, 2 * BG : 3 * BG], op0=ADD, op1=MUL)
                        nc.vector.scalar_tensor_tensor(
                            out=vv[g][D4:, :], in0=t4[g][D4:, :BG], scalar=1.0,
                            in1=t4[g][D4:, 3 * BG :], op0=SUB, op1=MUL)
                    for g in range(GB):
                        nc.tensor.matmul(ps[g][D4:, 4 * BG : 5 * BG], blk(AIN), stks[g][:],
                                         start=True, stop=False, skip_group_check=True)
                        nc.tensor.matmul(ps[g][D4:, 4 * BG : 5 * BG],
                                         w["w_ids"][D4:, D : 2 * D], uu[g][D4:, :],
                                         start=False, stop=False, skip_group_check=True)
                        nc.tensor.matmul(ps[g][D4:, 4 * BG : 5 * BG],
                                         w["w_ids"][D4:, 2 * D :], vv[g][D4:, :],
                                         start=False, stop=True, skip_group_check=True)
                    for g in range(GB):
                        nc.scalar.activation(out=ta[g][D4:, :], in_=ps[g][D4:, 4 * BG : 5 * BG],
                                             func=TANH)
                    for g in range(GB):  # A_new(2x) = (a'+1)*tanh(A_input)
                        nc.vector.scalar_tensor_tensor(
                            out=stks[g][D4:, :], in0=t4[g][D4:, BG : 2 * BG], scalar=1.0,
                            in1=ta[g][D4:, :], op0=ADD, op1=MUL)
                    for g in range(GB):
                        nc.tensor.matmul(pp[g * BG : (g + 1) * BG, k * O : (k + 1) * O],
                                         stks[g][D4:, :], w["w_phi"][D4:, :],
                                         start=True, stop=True, skip_group_check=True)

                nc.vector.tensor_add(ot[:], pp[:], w["phib"][:])
                nc.sync.dma_start(out=out[:, ds(it, E), :],
                                  in_=ot[:].rearrange("b (e o) -> b e o", o=O))

    nc.finalize()
    return nc


def _run(nc, in_maps, trace):
    from concourse.bass_utils import run_bass_kernel_spmd

    if trace:
        import sys as _sys, types as _types

        try:
            import antenv.axon_hooks  # noqa: F401
        except ImportError:
            import trn_agent_boot.trn_boot as _tb

            _hook = _tb._ntff_profile_via_ctypes("/opt/axon/libaxon_pjrt.so")
            _m = _types.ModuleType("antenv.axon_hooks")
            _m.get_axon_ntff_profile_hook = lambda: _hook
            _sys.modules["antenv.axon_hooks"] = _m
    return run_bass_kernel_spmd(nc, in_maps, core_ids=list(range(NCORES)), trace=trace)


def kernel(x, params, _trace=False, _t_steps=T):
    x = np.asarray(x)
    assert x.shape[0] == B and x.shape[2] == S, x.shape
    x = x[:, :_t_steps]
    pk = _prep_params(params)
    nc = _build(_t_steps)

    in_maps = []
    for c in range(NCORES):
        xc = x[c * BC : (c + 1) * BC].transpose(2, 1, 0)  # (S, t, BC)
        xc = np.concatenate([xc, np.ones((1, _t_steps, BC), np.float32)], axis=0)
        m = {"x": np.ascontiguousarray(xc.astype(BF16))}
        m.update(pk)
        in_maps.append(m)

    res = _run(nc, in_maps, _trace)
    kernel._last_results = res
    full = np.empty((B, _t_steps, O), np.float32)
    for c in range(NCORES):
        full[c * BC : (c + 1) * BC] = res.results[c]["out"]
    return full
